# revision 1
# baseline (speedup 1.0000x reference)
"""AdaptiveRSNNEmbedding Trainium2 kernel (8 NeuronCores, batch-parallel).

Reference semantics (per batch element, T time-reversed steps):
    g, c   = split(conv3x3(spike_prev, w_gate) + conv3x3(ev_t, w_in) + biases)
    gate   = sigmoid(g);  v = gate*vmem + c
    spike  = (v > 0.5);   vmem' = v - 0.5*spike
    agg[seg] accumulates vavg at spikes (seg<4), seg += spike, plus a final
    flush of the unclosed segment.

Key identity used here: the whole vavg/scatter/final-flush logic is exactly
    agg[s] += v_t * (seg_t == s)        (seg_t = #spikes before step t, uncapped)
applied every step, which removes vavg and the final pass entirely.

Device layout (per core, per batch element, processed sequentially):
  state partition p = 32*hb + c (hb = H/4 row block, c = channel), free = r*W+x
  spk (ping-pong pair) [128, (BR+2)*(W+2)]: 0.5*spike, padded cols +
      duplicated guard rows so conv taps never cross partitions (w_gate is
      pre-doubled on host). Double-buffered across steps because chunks of
      step t would otherwise overwrite rows step t's later chunks still read.
  conv = 4 accumulating matmuls per 3-row slice: 3x K=96 (dy-stacked spikes
      via one overlapping-window DMA, dx via AP col offset) + 1x K=18
      (host-prestacked event taps).
  agg[3] lives in DRAM with streamed read-modify-write to free SBUF for
      double buffering.
"""
import sys
import time
import numpy as np

if '/opt/trn_rl_repo' not in sys.path:
    sys.path.insert(0, '/opt/trn_rl_repo')

import concourse.bass as bass
import concourse.mybir as mybir
from concourse.tile import TileContext

F32 = mybir.dt.float32
AF = mybir.ActivationFunctionType
OP = mybir.AluOpType

# problem constants
B, T, CIN, COUT, H, W = 16, 16, 2, 32, 160, 160
TSEG = 4
NCORES = 8
BL = B // NCORES
THRESH = 0.5

CONV_DT = F32  # float32 (exact, slow PE) | float32r (fast, validate on HW)
F32R = mybir.dt.float32r
CONV_MODE = "f32r2"   # "f32": all-f32 matmuls | "f32r2": gate conv as hi+lo f32r


def f32r_round(x):
    """Round f32 to 11 explicit mantissa bits (RNE) - matches HW f32r."""
    u = np.ascontiguousarray(x, np.float32).view(np.uint32)
    sh = 12
    bias = ((u >> sh) & 1) + np.uint32((1 << (sh - 1)) - 1)
    u = (u + bias) & ~np.uint32((1 << sh) - 1)
    return u.view(np.float32)


def build_nc(BL=BL, T=T, H=H, W=W, conv_dt=F32, mode=None):
    if mode is None:
        mode = CONV_MODE
    spk_dt = F32R if mode == "f32r2" else conv_dt
    HB = 4
    BR = H // HB              # rows per block
    NPIX = BR * W             # per-partition state pixels
    WP = W + 2                # padded spike width
    SR = max(1, 512 // W)     # matmul slice rows (psum bank limit)
    CRMAX = 2 * SR            # chunk rows (2 slices/chunk -> 2-bank psum tiles)
    CF = CRMAX * W            # chunk free size

    nc = bass.Bass()
    nop_sem = nc.semaphore("nopsem").__enter__()  # sink for wait-split nops
    ev_d = nc.declare_dram_parameter("ev", [BL * T, 32, H * W], conv_dt,
                                     isOutput=False)
    WWC = 448 if mode == "f32r2" else 256
    ww_d = nc.declare_dram_parameter("ww", [128, WWC], conv_dt, isOutput=False)
    bgc_d = nc.declare_dram_parameter("bgc", [32, 2], F32, isOutput=False)
    out_d = nc.declare_dram_parameter("out", [TSEG, BL, COUT, H, W], F32,
                                      isOutput=True)
    a3_d = nc.dram_tensor("a3buf", [BL, 128, NPIX], F32)

    chunks = []
    r = 0
    while r < BR:
        cr = min(CRMAX, BR - r)
        chunks.append((r, cr))
        r += cr

    with TileContext(nc) as tc:
        with tc.tile_pool(name="const", bufs=1) as cpool, \
             tc.tile_pool(name="state", bufs=1) as spool, \
             tc.tile_pool(name="wss", bufs=2) as sspool, \
             tc.tile_pool(name="wev", bufs=2) as evpool, \
             tc.tile_pool(name="w1", bufs=2) as w1pool, \
             tc.tile_pool(name="wa3", bufs=2) as a3pool, \
             tc.tile_pool(name="psum", bufs=4, space="PSUM") as ppool:

            ww_t = cpool.tile([128, WWC], conv_dt, tag="ww")
            nc.sync.dma_start(out=ww_t[:], in_=ww_d[:])
            if mode == "f32r2":
                ww_r = cpool.tile([128, 384], F32R, tag="wwr")
                nc.scalar.activation(ww_r[0:96, :], ww_t[0:96, 0:384],
                                     AF.Copy)
                wg = [ww_r[0:96, 64 * kx:64 * (kx + 1)] for kx in range(6)]
                wi_aps = [ww_t[32 * hb:32 * (hb + 1), 384:448]
                          for hb in range(4)]
            else:
                wg = [ww_t[0:96, 64 * kx:64 * (kx + 1)] for kx in range(3)]
                wi_aps = [ww_t[32 * hb:32 * (hb + 1), 192:256]
                          for hb in range(4)]
            bgc_t = cpool.tile([32, 2], F32, tag="bgc")
            nc.sync.dma_start(out=bgc_t[:], in_=bgc_d[:])
            bg_t = bgc_t[:, 0:1]
            bc_t = bgc_t[:, 1:2]

            vmem = spool.tile([128, NPIX], F32, tag="vmem")
            seg = spool.tile([128, NPIX], mybir.dt.uint8, tag="seg")  # #spikes
            aggs = [spool.tile([128, NPIX], F32, tag=f"agg{s}", name=f"agg{s}")
                    for s in range(3)]
            spkbufs = [spool.tile([128, (BR + 2) * WP], spk_dt, tag=f"spk{i}",
                                  name=f"spk{i}") for i in range(2)]

            nc.nop_sem_num = nop_sem.num

            for b in range(BL):
                # zero state
                nc.vector.memset(vmem[:], 0.0)
                nc.vector.memset(seg[:], 0.0)
                for s in range(3):
                    nc.gpsimd.memset(aggs[s][:], 0.0)
                nc.gpsimd.memset(spkbufs[0][:].bitcast(F32), 0.0)
                nc.gpsimd.memset(spkbufs[1][:].bitcast(F32), 0.0)

                for t in range(T):
                    spk = spkbufs[t % 2]         # previous step's spikes (read)
                    spkw = spkbufs[(t + 1) % 2]  # this step's spikes (write)
                    for (r0, cr) in chunks:
                        F = cr * W
                        sl = slice(r0 * W, r0 * W + F)
                        nsl = (cr + SR - 1) // SR
                        # events for all 4 row blocks in one DMA
                        ev_t = evpool.tile([128, CF], conv_dt, tag="ev")
                        for g in range(4):
                            nc.sync.dma_start(
                                out=ev_t[32 * g:32 * (g + 1), :F],
                                in_=ev_d[b * T + t, :,
                                         (g * BR + r0) * W:(g * BR + r0) * W + F])

                        ps_ts = []
                        for hb in range(4):
                            # dy-stacked spikes (ky-major): 3 plain DMAs
                            ss = sspool.tile([96, CRMAX * WP], spk_dt, tag="ss")
                            for ky in range(3):
                                nc.sync.dma_start(
                                    out=ss[32 * ky:32 * (ky + 1), :cr * WP],
                                    in_=spk[32 * hb:32 * (hb + 1),
                                            (r0 + ky) * WP:(r0 + ky + cr) * WP])
                            ps = ppool.tile([64, nsl * 512], F32, tag="ps")
                            ps_ts.append(ps)
                            pbase = 0
                            tp = None
                            ss_r = ss[:].rearrange("k (r c) -> k r c", c=WP)
                            nr, isl = 0, 0
                            while nr < cr:
                                srr = min(SR, cr - nr)
                                out_ap = ps[pbase:pbase + 64,
                                            isl * 512:isl * 512 + srr * W]
                                ngm = 6 if mode == "f32r2" else 3
                                for kx in range(ngm):
                                    nc.tensor.matmul(
                                        out_ap, wg[kx],
                                        ss_r[:, nr:nr + srr,
                                             kx % 3:kx % 3 + W],
                                        start=(kx == 0), stop=False,
                                        tile_position=tp)
                                nc.tensor.matmul(
                                    out_ap, wi_aps[hb],
                                    ev_t[32 * hb:32 * (hb + 1),
                                         nr * W:(nr + srr) * W],
                                    start=False, stop=True,
                                    tile_position=(32 * hb, 0))
                                nr += srr
                                isl += 1

                        # psum -> sbuf extraction (cross-partition-base ACT)
                        gate_t = w1pool.tile([128, CF], F32, tag="gate")
                        cur_t = w1pool.tile([128, CF], F32, tag="cur")
                        for hb in range(4):
                            ps = ps_ts[hb]
                            if cr == nsl * SR:
                                ps_g = ps[0:32, :].rearrange(
                                    "p (n x) -> p n x", x=512)[:, 0:nsl, 0:SR * W]
                                ps_c = ps[32:64, :].rearrange(
                                    "p (n x) -> p n x", x=512)[:, 0:nsl, 0:SR * W]
                                g_o = gate_t[32 * hb:32 * (hb + 1), :F].rearrange(
                                    "p (n x) -> p n x", x=SR * W)
                                c_o = cur_t[32 * hb:32 * (hb + 1), :F].rearrange(
                                    "p (n x) -> p n x", x=SR * W)
                                nc.scalar.activation(g_o, ps_g, AF.Sigmoid,
                                                     bias=bg_t)
                                nc.scalar.activation(c_o, ps_c, AF.Identity,
                                                     bias=bc_t)
                            else:
                                nr, isl = 0, 0
                                while nr < cr:
                                    srr = min(SR, cr - nr)
                                    o0, o1 = nr * W, (nr + srr) * W
                                    p0 = isl * 512
                                    nc.scalar.activation(
                                        gate_t[32 * hb:32 * (hb + 1), o0:o1],
                                        ps[0:32, p0:p0 + srr * W],
                                        AF.Sigmoid, bias=bg_t)
                                    nc.scalar.activation(
                                        cur_t[32 * hb:32 * (hb + 1), o0:o1],
                                        ps[32:64, p0:p0 + srr * W],
                                        AF.Identity, bias=bc_t)
                                    nr += srr
                                    isl += 1

                        # state update
                        v_t = w1pool.tile([128, CF], F32, tag="v")
                        nc.vector.tensor_tensor(v_t[:, :F], gate_t[:, :F],
                                                vmem[:, sl], OP.mult)
                        nc.vector.tensor_tensor(v_t[:, :F], v_t[:, :F],
                                                cur_t[:, :F], OP.add)
                        # 0.5*spike -> spkw (strided padded rows)
                        spk_sl = spkw[:].rearrange("p (r c) -> p r c", c=WP)[
                            :, r0 + 1:r0 + 1 + cr, 1:1 + W]
                        spk_rd = spk_sl.bitcast(F32) if mode == "f32r2" \
                            else spk_sl
                        nc.vector.tensor_scalar(spk_sl, v_t[:, :F], THRESH, 0.5,
                                                OP.is_gt, OP.mult)
                        # guard-row duplication across block boundaries
                        spk_r = spkw[:].rearrange("p (r c) -> p r c", c=WP)
                        if r0 == 0:
                            for h in range(3):
                                nc.vector.tensor_scalar(
                                    spk_r[32 * h:32 * (h + 1), BR + 1, 1:1 + W],
                                    v_t[32 * (h + 1):32 * (h + 2), 0:W],
                                    THRESH, 0.5, OP.is_gt, OP.mult)
                        if r0 + cr == BR:
                            for h in range(3):
                                nc.vector.tensor_scalar(
                                    spk_r[32 * (h + 1):32 * (h + 2), 0, 1:1 + W],
                                    v_t[32 * h:32 * (h + 1),
                                        (cr - 1) * W:cr * W],
                                    THRESH, 0.5, OP.is_gt, OP.mult)
                        # vmem' = v - 0.5*spike
                        nc.vector.tensor_tensor(vmem[:, sl], v_t[:, :F],
                                                spk_rd, OP.subtract)
                        # agg[s] += v * (seg == s); s=3 streams through DRAM
                        a3t = a3pool.tile([128, CF], F32, tag="a3")
                        if t > 0:
                            nc.sync.dma_start(out=a3t[:, :F],
                                              in_=a3_d[b, :, sl])
                        for s in range(TSEG):
                            mv = gate_t if s % 2 == 0 else cur_t
                            if s == 3 and t == 0:
                                mv = a3t
                            nc.vector.scalar_tensor_tensor(
                                mv[:, :F], seg[:, sl], float(s), v_t[:, :F],
                                OP.is_equal, OP.mult)
                            if s < 3:
                                eng = nc.vector if s < 2 else nc.gpsimd
                                eng.tensor_tensor(aggs[s][:, sl],
                                                  aggs[s][:, sl],
                                                  mv[:, :F], OP.add)
                            elif t > 0:
                                nc.gpsimd.tensor_tensor(a3t[:, :F], a3t[:, :F],
                                                        mv[:, :F], OP.add)
                        nc.sync.dma_start(out=a3_d[b, :, sl], in_=a3t[:, :F])
                        # seg += spike (after agg reads); spk holds 0.5*spike
                        nc.vector.scalar_tensor_tensor(
                            seg[:, sl], spk_rd, 2.0, seg[:, sl], OP.mult,
                            OP.add)

                # write this batch element's aggregation out
                for s in range(3):
                    for hb in range(4):
                        nc.sync.dma_start(
                            out=out_d[s, b, :, hb * BR:(hb + 1) * BR, :],
                            in_=aggs[s][32 * hb:32 * (hb + 1), :])
                for hb in range(4):
                    nc.sync.dma_start(
                        out=out_d[3, b, :, hb * BR:(hb + 1) * BR, :],
                        in_=a3_d[b, 32 * hb:32 * (hb + 1), :])
    _split_matmul_waits(nc)
    return nc


def _split_matmul_waits(nc):
    """Walrus's LDW+MATMUL pair (and 2D DMA descriptors) have a single
    sync-wait slot; move extra waits onto same-engine no-ops inserted just
    before the instruction (safe: waits execute in order on the sequencer)."""
    nid = [0]
    for blk in nc.m.functions[0].blocks:
        out = []
        for inst in blk.instructions:
            si = inst.sync_info
            if (type(inst).__name__ != 'InstNoOp' and si is not None
                    and len(si.on_wait) > 1):
                keep = si.on_wait[-1:]
                for w in si.on_wait[:-1]:
                    nop = mybir.InstNoOp(name=f"NW-{nid[0]}", ins=[], outs=[])
                    nid[0] += 1
                    nop.engine = inst.engine
                    zupd = mybir.SyncUpdate(
                        sync_type='semaphore', id=nc.nop_sem_num,
                        ant_name='nopsem', update_mode='sem-inc',
                        update_value=1, update_reg=None)
                    nop.sync_info = mybir.SyncInfo(on_wait=[w],
                                                   on_update=[zupd])
                    out.append(nop)
                inst.sync_info = mybir.SyncInfo(on_wait=keep,
                                                on_update=si.on_update)
            out.append(inst)
        blk.instructions = out


def host_prep(events, w_in, b_in, w_gate, b_gate, conv_np=np.float32,
              ncores=NCORES, mode=None):
    if mode is None:
        mode = CONV_MODE
    """Build per-core input maps. events: [B,T,CIN,H,W] full."""
    Bf, Tf, Cf, Hf, Wf = events.shape
    # time reversal + zero pad + 3x3 tap stacking -> [B,T,18,H,W]
    evr = events[:, ::-1]
    evp = np.zeros((Bf, Tf, Cf, Hf + 2, Wf + 2), np.float32)
    evp[..., 1:1 + Hf, 1:1 + Wf] = evr
    win = np.lib.stride_tricks.sliding_window_view(evp, (3, 3), axis=(3, 4))
    # win: [B,T,C,H,W,3,3]; tap value for output (r,x) tap (ky,kx) is
    # ev[r+ky-1, x+kx-1] = win[..., r, x, ky, kx]; padded to 32 taps (the PE
    # reads full 32-row groups when tile_position is set)
    ev_st = np.zeros((Bf, Tf, 32, Hf * Wf), np.float32)
    ev_st[:, :, :18] = np.ascontiguousarray(
        win.transpose(0, 1, 2, 5, 6, 3, 4)).reshape(Bf, Tf, 18, Hf * Wf)
    ev_st = ev_st.astype(conv_np)

    # weights packed in one [128, 256] tile: wg rows (c*3+ky) cols (kx*64+m),
    # wi rows (cin*9+ky*3+kx) at cols 192:256; gate conv doubled (spk stores
    # 0.5*spike)
    WWC = 448 if mode == "f32r2" else 256
    wioff = 384 if mode == "f32r2" else 192
    ww = np.zeros((128, WWC), np.float32)
    for kx in range(3):
        for ky in range(3):
            for cin in range(COUT):
                wgv = 2.0 * w_gate[:, cin, ky, kx]
                if mode == "f32r2":
                    hi = f32r_round(wgv)
                    lo = f32r_round(wgv - hi)
                    ww[32 * ky + cin, 64 * kx:64 * (kx + 1)] = hi
                    ww[32 * ky + cin, 192 + 64 * kx:192 + 64 * (kx + 1)] = lo
                else:
                    ww[32 * ky + cin, 64 * kx:64 * (kx + 1)] = wgv
    for hb in range(4):
        for cin in range(CIN):
            for ky in range(3):
                for kx in range(3):
                    ww[32 * hb + cin * 9 + ky * 3 + kx,
                       wioff:wioff + 64] = w_in[:, cin, ky, kx]
    bgc = np.stack([b_gate[:32] + b_in[:32], b_gate[32:] + b_in[32:]],
                   axis=1).astype(np.float32)
    ww = ww.astype(conv_np)

    bl = Bf // ncores
    in_maps = []
    for i in range(ncores):
        ev_i = ev_st[i * bl:(i + 1) * bl].reshape(bl * Tf, 32, Hf * Wf)
        in_maps.append({"ev": np.ascontiguousarray(ev_i), "ww": ww,
                        "bgc": bgc})
    return in_maps


_cache = {}
last_run_info = {}


def kernel(events, w_in, b_in, w_gate, b_gate, trace=False):
    from concourse import bass_utils
    key = ("full", str(CONV_DT), CONV_MODE)
    if key not in _cache:
        _cache[key] = build_nc(conv_dt=CONV_DT)
    nc = _cache[key]
    in_maps = host_prep(np.asarray(events), np.asarray(w_in), np.asarray(b_in),
                        np.asarray(w_gate), np.asarray(b_gate))
    t0 = time.time()
    res = bass_utils.run_bass_kernel_spmd(
        nc, in_maps, core_ids=list(range(NCORES)), trace=trace)
    wall = time.time() - t0
    last_run_info.update(exec_time_ns=res.exec_time_ns, wall_s=wall,
                         profile_json=getattr(res, "profile_json", None))
    outs = [res.results[i]["out"] for i in range(NCORES)]
    return np.concatenate(outs, axis=1)



# revision 12
# speedup vs baseline: 2.1262x; 2.1262x over previous
"""AdaptiveRSNNEmbedding Trainium2 kernel (8 NeuronCores, batch-parallel).

Reference semantics (per batch element, T time-reversed steps):
    g, c   = split(conv3x3(spike_prev, w_gate) + conv3x3(ev_t, w_in) + biases)
    gate   = sigmoid(g);  v = gate*vmem + c
    spike  = (v > 0.5);   vmem' = v - 0.5*spike
    agg[seg] accumulates vavg at spikes (seg<4), seg += spike, plus a final
    flush of the unclosed segment.

v2 design:
  * agg via cumulative-sum captures: S_t = sum v_tau; C[s] = S at the spike
    closing segment s; final C'[s] = (seg>s) ? C[s] : S_end;
    agg[s] = C'[s]-C'[s-1]. Captures are copy_predicated writes (4/step).
  * spikes stored as +-1 (ACT Sign(v-0.5)) with -1 padding; conv identity
    conv(h) = 0.5*conv(pm) + 0.5*sum(w) makes interior AND edges exact with
    the constant folded into the per-channel extraction bias.
  * conv in fp16 hi+lo passes (exact to ~22 bits): per kx, mm_hi uses
    [wg_hi | wi_hi(ev_hi rows) | wi_lo(ev_lo rows)] and mm_lo uses
    [wg_lo | wi_lo(ev_hi rows) | wi_hi(ev_lo rows)] so the event product
    (ev_hi+ev_lo)*(wi_hi+wi_lo) is complete. Events ride as 12 extra K rows
    (2cin x 3ky x hi/lo) in padded spatial layout, ky pre-shifted on host.
  * layout: partition p = 32*hb + c (hb = H/4 row block), free = r*W+x.
    psum pair tiles [128, nsl*512] hold 2 row blocks (hb pair) per chunk.
  * state updates at 2-conv-chunk granularity; vmem/seg on Pool engine;
    S/C/seg/masks/spk in fp16 (exact or validated), v path in f32.
"""
import sys
import time
import numpy as np

if '/opt/trn_rl_repo' not in sys.path:
    sys.path.insert(0, '/opt/trn_rl_repo')

import concourse.bass as bass
import concourse.mybir as mybir
from concourse.tile import TileContext

F32 = mybir.dt.float32
F16 = mybir.dt.float16
AF = mybir.ActivationFunctionType
OP = mybir.AluOpType

B, T, CIN, COUT, H, W = 16, 16, 2, 32, 160, 160
TSEG = 4
NCORES = 8
BL = B // NCORES
THRESH = 0.5
HB = 4


def _chunks(BR, CR):
    out = []
    r = 0
    while r < BR:
        cr = min(CR, BR - r)
        out.append((r, cr))
        r += cr
    return out


def build_nc(BL=BL, T=T, H=H, W=W, debug=False):
    BR = H // HB
    NPIX = BR * W
    WP = W + 2
    SR = max(1, 512 // W)          # rows per psum slice
    CRMAX = 2 * SR                 # conv chunk rows
    cchunks = _chunks(BR, CRMAX)
    # state chunks = pairs of conv chunks
    schunks = []
    i = 0
    while i < len(cchunks):
        r0 = cchunks[i][0]
        rows = cchunks[i][1]
        if i + 1 < len(cchunks) and cchunks[i + 1][1] == CRMAX:
            rows += cchunks[i + 1][1]
            i += 2
        else:
            i += 1
        schunks.append((r0, rows))
    SFMAX = max(rows for _, rows in schunks) * W

    nc = bass.Bass()
    nop_sem = nc.semaphore("nopsem").__enter__()
    ev_d = nc.declare_dram_parameter("ev", [BL * T, HB, 12, BR * WP], F16,
                                     isOutput=False)
    ww_d = nc.declare_dram_parameter("ww", [128, 384], F16, isOutput=False)
    bias_d = nc.declare_dram_parameter("bias", [128, 2], F32, isOutput=False)
    out_d = nc.declare_dram_parameter("out", [TSEG, BL, COUT, H, W], F32,
                                      isOutput=True)
    if debug:
        dbg_d = nc.declare_dram_parameter("dbg", [4, 128, (H // HB) * W],
                                          F32, isOutput=True)

    with TileContext(nc) as tc:
        with tc.tile_pool(name="const", bufs=1) as cpool, \
             tc.tile_pool(name="state", bufs=1) as spool, \
             tc.tile_pool(name="wss", bufs=2) as sspool, \
             tc.tile_pool(name="wgc", bufs=1) as gcpool, \
             tc.tile_pool(name="wst", bufs=2) as stpool, \
             tc.tile_pool(name="wm", bufs=2) as mpool, \
             tc.tile_pool(name="psum", bufs=2, space="PSUM") as ppool:

            ww_t = cpool.tile([128, 384], F16, tag="ww")
            nc.sync.dma_start(out=ww_t[:], in_=ww_d[:])
            bias_t = cpool.tile([128, 2], F32, tag="bias")
            nc.sync.dma_start(out=bias_t[:], in_=bias_d[:])

            vmem = spool.tile([128, NPIX], F32, tag="vmem")
            S_t = spool.tile([128, NPIX], F16, tag="S")
            seg = spool.tile([128, NPIX], F16, tag="seg")
            C_ts = [spool.tile([128, NPIX], F16, tag=f"C{s}", name=f"C{s}")
                    for s in range(TSEG)]
            spkbufs = [spool.tile([128, (BR + 2) * WP], F16, tag=f"spk{i}",
                                  name=f"spk{i}") for i in range(2)]

            nc.nop_sem_num = nop_sem.num

            for b in range(BL):
                # zero/reset state
                nc.vector.memset(vmem[:], 0.0)
                nc.vector.memset(S_t[:], 0.0)
                nc.vector.memset(seg[:], 0.0)
                for s in range(TSEG):
                    nc.gpsimd.memset(C_ts[s][:], 0.0)
                nc.gpsimd.memset(spkbufs[0][:], -1.0)
                nc.gpsimd.memset(spkbufs[1][:], -1.0)

                for t in range(T):
                    spk = spkbufs[t % 2]
                    spkw = spkbufs[(t + 1) % 2]
                    bt = b * T + t

                    for (sr0, srows) in schunks:
                        # ---- conv phase for this state chunk ----
                        gt = gcpool.tile([128, SFMAX], F32, tag="gate",
                                         name="gt")
                        ct = gcpool.tile([128, SFMAX], F32, tag="cur",
                                         name="ct")
                        for (r0, cr) in [c for c in cchunks
                                         if sr0 <= c[0] < sr0 + srows]:
                            F = cr * W
                            nsl = (cr + SR - 1) // SR
                            # stack spikes + events per hb
                            ss_ts = []
                            for hb in range(HB):
                                ss = sspool.tile([128, CRMAX * WP], F16,
                                                 tag=f"ss{hb}",
                                                 name=f"ss{hb}")
                                ss_ts.append(ss)
                                for ky in range(3):
                                    nc.sync.dma_start(
                                        out=ss[32 * ky:32 * (ky + 1),
                                               :cr * WP],
                                        in_=spk[32 * hb:32 * (hb + 1),
                                                (r0 + ky) * WP:
                                                (r0 + ky + cr) * WP])
                                nc.sync.dma_start(
                                    out=ss[96:108, :cr * WP],
                                    in_=ev_d[bt, hb, :,
                                             r0 * WP:(r0 + cr) * WP])
                            ps01 = ppool.tile([128, 1024], F32,
                                              tag="ps01", name="ps01")
                            ps23 = ppool.tile([128, 1024], F32,
                                              tag="ps23", name="ps23")
                            pst = [ps01, ps01, ps23, ps23]
                            for ipass in range(6):  # (hi,lo) x kx
                                wap = ww_t[0:108, 64 * ipass:64 * ipass + 64]
                                kx = ipass % 3
                                first = ipass == 0
                                last = ipass == 5
                                for hb in range(HB):
                                    ps = pst[hb]
                                    half = hb % 2
                                    ss_r = ss_ts[hb][:].rearrange(
                                        "k (r c) -> k r c", c=WP)
                                    nr, isl = 0, 0
                                    while nr < cr:
                                        srr = min(SR, cr - nr)
                                        out_ap = ps[64 * half:64 * half + 64,
                                                    isl * 512:
                                                    isl * 512 + srr * W]
                                        nc.tensor.matmul(
                                            out_ap, wap,
                                            ss_r[0:108, nr:nr + srr,
                                                 kx:kx + W],
                                            start=first, stop=last,
                                            skip_group_check=True)
                                        nr += srr
                                        isl += 1
                            # extraction: gate=sigmoid(ps+bg), cur=ps+bc
                            coff = (r0 - sr0) * W
                            bgap = bias_t[0:32, 0:1]
                            bcap = bias_t[32:64, 0:1]
                            for hb in range(HB):
                                ps = pst[hb]
                                h0 = 64 * (hb % 2)
                                if cr == nsl * SR:
                                    ps_g = ps[h0:h0 + 32, :].rearrange(
                                        "p (n x) -> p n x", x=512)[
                                        :, 0:nsl, 0:SR * W]
                                    ps_c = ps[h0 + 32:h0 + 64, :].rearrange(
                                        "p (n x) -> p n x", x=512)[
                                        :, 0:nsl, 0:SR * W]
                                    go = gt[32 * hb:32 * (hb + 1),
                                            coff:coff + F].rearrange(
                                        "p (n x) -> p n x", x=SR * W)
                                    co = ct[32 * hb:32 * (hb + 1),
                                            coff:coff + F].rearrange(
                                        "p (n x) -> p n x", x=SR * W)
                                    nc.scalar.activation(go, ps_g,
                                                         AF.Sigmoid,
                                                         bias=bgap)
                                    nc.scalar.activation(co, ps_c,
                                                         AF.Identity,
                                                         bias=bcap)
                                else:
                                    nr, isl = 0, 0
                                    while nr < cr:
                                        srr = min(SR, cr - nr)
                                        o0, o1 = (coff + nr * W,
                                                  coff + (nr + srr) * W)
                                        p0 = isl * 512
                                        nc.scalar.activation(
                                            gt[32 * hb:32 * (hb + 1),
                                               o0:o1],
                                            ps[h0:h0 + 32,
                                               p0:p0 + srr * W],
                                            AF.Sigmoid, bias=bgap)
                                        nc.scalar.activation(
                                            ct[32 * hb:32 * (hb + 1),
                                               o0:o1],
                                            ps[h0 + 32:h0 + 64,
                                               p0:p0 + srr * W],
                                            AF.Identity, bias=bcap)
                                        nr += srr
                                        isl += 1

                        # ---- state phase for this state chunk ----
                        F = srows * W
                        sl = slice(sr0 * W, sr0 * W + F)
                        v_t = stpool.tile([128, SFMAX], F32, tag="v",
                                          name="v_t")
                        nc.vector.tensor_tensor(v_t[:, :F], gt[:, :F],
                                                vmem[:, sl], OP.mult)
                        nc.vector.tensor_tensor(v_t[:, :F], v_t[:, :F],
                                                ct[:, :F], OP.add)
                        # spikes: +-1 via ACT Sign(v-0.5), strided write
                        spk_sl = spkw[:].rearrange("p (r c) -> p r c",
                                                   c=WP)[
                            :, sr0 + 1:sr0 + 1 + srows, 1:1 + W]
                        nc.scalar.activation(spk_sl,
                                             v_t[:, :F].rearrange(
                                                 "p (r c) -> p r c", c=W),
                                             AF.Sign,
                                             bias=bias_t[:, 1:2])
                        # u = -0.25*pm - 0.25  (= -0.5*spike)
                        u_t = stpool.tile([128, SFMAX], F16, tag="u",
                                          name="u_t")
                        nc.scalar.activation(u_t[:, :F].rearrange(
                            "p (r c) -> p r c", c=W), spk_sl,
                            AF.Copy, bias=-0.25, scale=-0.25)
                        # S += v
                        nc.vector.tensor_tensor(S_t[:, sl], S_t[:, sl],
                                                v_t[:, :F], OP.add)
                        # key = seg - u (= seg + 0.5*spike)
                        key_t = stpool.tile([128, SFMAX], F16, tag="key",
                                            name="key_t")
                        nc.vector.tensor_tensor(key_t[:, :F], seg[:, sl],
                                                u_t[:, :F], OP.subtract)
                        # vmem = v + u ; seg = key - u   (Pool engine)
                        nc.gpsimd.tensor_tensor(vmem[:, sl], v_t[:, :F],
                                                u_t[:, :F], OP.add)
                        nc.gpsimd.tensor_tensor(seg[:, sl], key_t[:, :F],
                                                u_t[:, :F], OP.subtract)
                        # captures
                        for s in range(min(t + 1, TSEG)):
                            m_t = mpool.tile([128, SFMAX], mybir.dt.uint16,
                                             tag="m", name="m_t")
                            nc.vector.tensor_scalar(m_t[:, :F], key_t[:, :F],
                                                    s + 0.5, None,
                                                    OP.is_equal)
                            nc.vector.copy_predicated(C_ts[s][:, sl],
                                                      m_t[:, :F],
                                                      S_t[:, sl])
                        # guard rows for next step's conv halo
                        spk_r = spkw[:].rearrange("p (r c) -> p r c", c=WP)
                        if sr0 == 0:
                            # bottom guard of block h = first row of block h+1
                            for h in range(HB - 1):
                                nc.vector.tensor_copy(
                                    out=spk_r[32 * h:32 * (h + 1),
                                              BR + 1, 1:1 + W],
                                    in_=spk_r[32 * (h + 1):32 * (h + 2),
                                              1, 1:1 + W])
                        if sr0 + srows == BR:
                            # top guard of block h+1 = last row of block h
                            for h in range(HB - 1):
                                nc.vector.tensor_copy(
                                    out=spk_r[32 * (h + 1):32 * (h + 2),
                                              0, 1:1 + W],
                                    in_=spk_r[32 * h:32 * (h + 1),
                                              BR, 1:1 + W])

                if debug and b == 0:
                    dbgt = gcpool.tile([128, SFMAX], F32, tag="gate",
                                       name="dbgt")
                    nc.vector.tensor_copy(out=dbgt[:, :NPIX], in_=vmem[:])
                    nc.sync.dma_start(out=dbg_d[0], in_=dbgt[:, :NPIX])
                    nc.vector.tensor_copy(out=dbgt[:, :NPIX], in_=S_t[:])
                    nc.sync.dma_start(out=dbg_d[1], in_=dbgt[:, :NPIX])
                    nc.vector.tensor_copy(out=dbgt[:, :NPIX], in_=seg[:])
                    nc.sync.dma_start(out=dbg_d[2], in_=dbgt[:, :NPIX])
                    nc.vector.tensor_copy(out=dbgt[:, :NPIX],
                                          in_=C_ts[0][:])
                    nc.sync.dma_start(out=dbg_d[3], in_=dbgt[:, :NPIX])

                # ---- final flush + diffs + output ----
                for (sr0, srows) in schunks:
                    F = srows * W
                    sl = slice(sr0 * W, sr0 * W + F)
                    for s in range(TSEG):
                        mf = mpool.tile([128, SFMAX], mybir.dt.uint16,
                                        tag="m", name="mf")
                        nc.vector.tensor_scalar(mf[:, :F], seg[:, sl],
                                                s + 0.5, None, OP.is_lt)
                        nc.vector.copy_predicated(C_ts[s][:, sl],
                                                  mf[:, :F], S_t[:, sl])
                    for s in range(TSEG):
                        og = gcpool.tile([128, SFMAX], F32, tag="gate",
                                         name="og")
                        if s == 0:
                            nc.vector.tensor_copy(out=og[:, :F],
                                                  in_=C_ts[0][:, sl])
                        else:
                            nc.vector.tensor_tensor(og[:, :F],
                                                    C_ts[s][:, sl],
                                                    C_ts[s - 1][:, sl],
                                                    OP.subtract)
                        for hb in range(HB):
                            nc.sync.dma_start(
                                out=out_d[s, b, :,
                                          hb * BR + sr0:
                                          hb * BR + sr0 + srows, :],
                                in_=og[32 * hb:32 * (hb + 1), :F])
    _split_matmul_waits(nc)
    return nc


def _split_matmul_waits(nc):
    """Walrus's LDW+MATMUL pair (and 2D DMA descriptors) have a single
    sync-wait slot; move extra waits onto same-engine no-ops inserted just
    before the instruction (safe: waits execute in order on the sequencer)."""
    nid = [0]
    for blk in nc.m.functions[0].blocks:
        out = []
        for inst in blk.instructions:
            si = inst.sync_info
            if (type(inst).__name__ != 'InstNoOp' and si is not None
                    and len(si.on_wait) > 1):
                keep = si.on_wait[-1:]
                for w in si.on_wait[:-1]:
                    nop = mybir.InstNoOp(name=f"NW-{nid[0]}", ins=[], outs=[])
                    nid[0] += 1
                    nop.engine = inst.engine
                    zupd = mybir.SyncUpdate(
                        sync_type='semaphore', id=nc.nop_sem_num,
                        ant_name='nopsem', update_mode='sem-inc',
                        update_value=1, update_reg=None)
                    nop.sync_info = mybir.SyncInfo(on_wait=[w],
                                                   on_update=[zupd])
                    out.append(nop)
                inst.sync_info = mybir.SyncInfo(on_wait=keep,
                                                on_update=si.on_update)
            out.append(inst)
        blk.instructions = out


def host_prep(events, w_in, b_in, w_gate, b_gate, ncores=NCORES):
    """Build per-core input maps. events: [B,T,CIN,H,W] full."""
    Bf, Tf, Cf, Hf, Wf = events.shape
    Cout2 = w_gate.shape[0]          # 64
    BR = Hf // HB
    WP = Wf + 2
    evr = np.ascontiguousarray(events[:, ::-1]).astype(np.float32)
    evh = evr.astype(np.float16)
    evl = (evr - evh.astype(np.float32)).astype(np.float16)
    # padded planes [B,T,2,H+2,WP]
    def padp(x):
        p = np.zeros((Bf, Tf, Cf, Hf + 2, WP), np.float16)
        p[..., 1:1 + Hf, 1:1 + Wf] = x
        return p
    evph, evpl = padp(evh), padp(evl)
    # ev_d [B,T,4,12,BR*WP]: row hl*6+ky*2+cin content r -> pad[g0+r+ky]
    ev_st = np.zeros((Bf, Tf, HB, 12, BR, WP), np.float16)
    for hl, src in ((0, evph), (1, evpl)):
        for ky in range(3):
            for cin in range(Cf):
                for hb in range(HB):
                    g0 = hb * BR
                    ev_st[:, :, hb, hl * 6 + ky * 2 + cin] = \
                        src[:, :, cin, g0 + ky:g0 + ky + BR, :]
    ev_st = ev_st.reshape(Bf, Tf, HB, 12, BR * WP)

    # weights: 6 tiles [108,64] packed in ww[128,384]
    wg = 0.5 * np.asarray(w_gate, np.float32)
    wgh = wg.astype(np.float16)
    wgl = (wg - wgh.astype(np.float32)).astype(np.float16)
    wi = np.asarray(w_in, np.float32)
    wih = wi.astype(np.float16)
    wil = (wi - wih.astype(np.float32)).astype(np.float16)
    ww = np.zeros((128, 384), np.float16)
    for ipass in range(6):
        hi = ipass < 3
        kx = ipass % 3
        c0 = 64 * ipass
        wgp = wgh if hi else wgl
        we1 = wih if hi else wil   # on ev_hi rows
        we2 = wil if hi else wih   # on ev_lo rows (cross)
        for ky in range(3):
            for c in range(COUT):
                ww[ky * 32 + c, c0:c0 + 64] = wgp[:, c, ky, kx]
            for cin in range(Cf):
                ww[96 + ky * 2 + cin, c0:c0 + 64] = we1[:, cin, ky, kx]
                ww[102 + ky * 2 + cin, c0:c0 + 64] = we2[:, cin, ky, kx]

    # bias: b + 0.5*sum(w_gate) per out channel; rows [bg,bc,bg,bc]x32
    bsum = 0.5 * np.asarray(w_gate, np.float32).sum(axis=(1, 2, 3))
    beff = (np.asarray(b_gate, np.float32) + np.asarray(b_in, np.float32)
            + bsum)                       # [64]
    bias = np.zeros((128, 2), np.float32)
    bias[:, 1] = -THRESH
    bias[0:32, 0] = beff[:32]
    bias[32:64, 0] = beff[32:]
    bias[64:96, 0] = beff[:32]
    bias[96:128, 0] = beff[32:]

    bl = Bf // ncores
    in_maps = []
    for i in range(ncores):
        ev_i = ev_st[i * bl:(i + 1) * bl].reshape(bl * Tf, HB, 12, BR * WP)
        in_maps.append({"ev": np.ascontiguousarray(ev_i), "ww": ww,
                        "bias": bias})
    return in_maps


_cache = {}
last_run_info = {}


def kernel(events, w_in, b_in, w_gate, b_gate, trace=False):
    from concourse import bass_utils
    key = ("v2",)
    if key not in _cache:
        _cache[key] = build_nc()
    nc = _cache[key]
    in_maps = host_prep(np.asarray(events), np.asarray(w_in),
                        np.asarray(b_in), np.asarray(w_gate),
                        np.asarray(b_gate))
    t0 = time.time()
    res = bass_utils.run_bass_kernel_spmd(
        nc, in_maps, core_ids=list(range(NCORES)), trace=trace)
    wall = time.time() - t0
    last_run_info.update(exec_time_ns=res.exec_time_ns, wall_s=wall,
                         profile_json=getattr(res, "profile_json", None))
    outs = [res.results[i]["out"] for i in range(NCORES)]
    return np.concatenate(outs, axis=1)


# revision 17
# speedup vs baseline: 2.2863x; 1.0753x over previous
"""AdaptiveRSNNEmbedding Trainium2 kernel (8 NeuronCores, batch-parallel).

Reference semantics (per batch element, T time-reversed steps):
    g, c   = split(conv3x3(spike_prev, w_gate) + conv3x3(ev_t, w_in) + biases)
    gate   = sigmoid(g);  v = gate*vmem + c
    spike  = (v > 0.5);   vmem' = v - 0.5*spike
    agg[seg] accumulates vavg at spikes (seg<4), seg += spike, plus a final
    flush of the unclosed segment.

v2 design:
  * agg via cumulative-sum captures: S_t = sum v_tau; C[s] = S at the spike
    closing segment s; final C'[s] = (seg>s) ? C[s] : S_end;
    agg[s] = C'[s]-C'[s-1]. Captures are copy_predicated writes (4/step).
  * spikes stored as +-1 (ACT Sign(v-0.5)) with -1 padding; conv identity
    conv(h) = 0.5*conv(pm) + 0.5*sum(w) makes interior AND edges exact with
    the constant folded into the per-channel extraction bias.
  * conv in fp16 hi+lo passes (exact to ~22 bits): per kx, mm_hi uses
    [wg_hi | wi_hi(ev_hi rows) | wi_lo(ev_lo rows)] and mm_lo uses
    [wg_lo | wi_lo(ev_hi rows) | wi_hi(ev_lo rows)] so the event product
    (ev_hi+ev_lo)*(wi_hi+wi_lo) is complete. Events ride as 12 extra K rows
    (2cin x 3ky x hi/lo) in padded spatial layout, ky pre-shifted on host.
  * layout: partition p = 32*hb + c (hb = H/4 row block), free = r*W+x.
    psum pair tiles [128, nsl*512] hold 2 row blocks (hb pair) per chunk.
  * state updates at 2-conv-chunk granularity; vmem/seg on Pool engine;
    S/C/seg/masks/spk in fp16 (exact or validated), v path in f32.
"""
import sys
import time
import numpy as np

if '/opt/trn_rl_repo' not in sys.path:
    sys.path.insert(0, '/opt/trn_rl_repo')

import concourse.bass as bass
import concourse.mybir as mybir
from concourse.tile import TileContext

F32 = mybir.dt.float32
F16 = mybir.dt.float16
AF = mybir.ActivationFunctionType
OP = mybir.AluOpType

B, T, CIN, COUT, H, W = 16, 16, 2, 32, 160, 160
TSEG = 4
NCORES = 8
BL = B // NCORES
THRESH = 0.5
HB = 4


def _chunks(BR, CR):
    out = []
    r = 0
    while r < BR:
        cr = min(CR, BR - r)
        out.append((r, cr))
        r += cr
    return out


def build_nc(BL=BL, T=T, H=H, W=W, debug=False):
    BR = H // HB
    NPIX = BR * W
    WP = W + 2
    SR = max(1, 512 // W)          # rows per psum slice
    CRMAX = 2 * SR                 # conv chunk rows
    cchunks = _chunks(BR, CRMAX)
    # state chunks = pairs of conv chunks
    schunks = []
    i = 0
    while i < len(cchunks):
        r0 = cchunks[i][0]
        rows = cchunks[i][1]
        if i + 1 < len(cchunks) and cchunks[i + 1][1] == CRMAX:
            rows += cchunks[i + 1][1]
            i += 2
        else:
            i += 1
        schunks.append((r0, rows))
    SFMAX = max(rows for _, rows in schunks) * W

    nc = bass.Bass()
    nop_sem = nc.semaphore("nopsem").__enter__()
    ev_d = nc.declare_dram_parameter("ev", [BL * T, HB, 12, BR * WP], F16,
                                     isOutput=False)
    ww_d = nc.declare_dram_parameter("ww", [128, 384], F16, isOutput=False)
    bias_d = nc.declare_dram_parameter("bias", [128, 2], F32, isOutput=False)
    out_d = nc.declare_dram_parameter("out", [TSEG, BL, COUT, H, W], F32,
                                      isOutput=True)
    if debug:
        dbg_d = nc.declare_dram_parameter("dbg", [4, 128, (H // HB) * W],
                                          F32, isOutput=True)

    with TileContext(nc) as tc:
        with tc.tile_pool(name="const", bufs=1) as cpool, \
             tc.tile_pool(name="state", bufs=1) as spool, \
             tc.tile_pool(name="wss", bufs=2) as sspool, \
             tc.tile_pool(name="wgc", bufs=1) as gcpool, \
             tc.tile_pool(name="wst", bufs=2) as stpool, \
             tc.tile_pool(name="wuk", bufs=1) as ukpool, \
             tc.tile_pool(name="wm", bufs=2) as mpool, \
             tc.tile_pool(name="psum", bufs=2, space="PSUM") as ppool:

            ww_t = cpool.tile([128, 384], F16, tag="ww")
            nc.sync.dma_start(out=ww_t[:], in_=ww_d[:])
            bias_t = cpool.tile([128, 2], F32, tag="bias")
            nc.sync.dma_start(out=bias_t[:], in_=bias_d[:])

            vmem = spool.tile([128, NPIX], F32, tag="vmem")
            S_t = spool.tile([128, NPIX], F16, tag="S")
            seg = spool.tile([128, NPIX], F16, tag="seg")
            C_ts = [spool.tile([128, NPIX], F16, tag=f"C{s}", name=f"C{s}")
                    for s in range(TSEG)]
            spkbufs = [spool.tile([128, (BR + 2) * WP], F16, tag=f"spk{i}",
                                  name=f"spk{i}") for i in range(2)]

            nc.nop_sem_num = nop_sem.num

            for b in range(BL):
                # zero/reset state
                nc.vector.memset(vmem[:], 0.0)
                nc.vector.memset(S_t[:], 0.0)
                nc.vector.memset(seg[:], 0.0)
                for s in range(TSEG):
                    nc.gpsimd.memset(C_ts[s][:], 0.0)
                nc.gpsimd.memset(spkbufs[0][:], -1.0)
                nc.gpsimd.memset(spkbufs[1][:], -1.0)

                for t in range(T):
                    spk = spkbufs[t % 2]
                    spkw = spkbufs[(t + 1) % 2]
                    bt = b * T + t

                    for (sr0, srows) in schunks:
                        # ---- conv phase for this state chunk ----
                        gt = gcpool.tile([128, SFMAX], F32, tag="gate",
                                         name="gt")
                        ct = gcpool.tile([128, SFMAX], F32, tag="cur",
                                         name="ct")
                        # stack spikes + events per hb for the whole sc
                        SCR = SFMAX // W
                        ss_ts = []
                        for hb in range(HB):
                            ss = sspool.tile([128, SCR * WP], F16,
                                             tag=f"ss{hb}", name=f"ss{hb}")
                            ss_ts.append(ss)
                            for ky in range(3):
                                hs = (hb + ky - 1) % HB
                                dlt = (hb + ky - 1) // HB
                                nc.sync.dma_start(
                                    out=ss[32 * ky:32 * (ky + 1),
                                           :srows * WP],
                                    in_=spk[32 * hs:32 * (hs + 1),
                                            (sr0 + dlt + 1) * WP:
                                            (sr0 + dlt + 1 + srows) * WP])
                            nc.sync.dma_start(
                                out=ss[96:108, :srows * WP],
                                in_=ev_d[bt, hb, :,
                                         sr0 * WP:(sr0 + srows) * WP])
                        for (r0, cr) in [c for c in cchunks
                                         if sr0 <= c[0] < sr0 + srows]:
                            F = cr * W
                            nsl = (cr + SR - 1) // SR
                            lr0 = r0 - sr0
                            ps01 = ppool.tile([128, 1024], F32,
                                              tag="ps01", name="ps01")
                            ps23 = ppool.tile([128, 1024], F32,
                                              tag="ps23", name="ps23")
                            pst = [ps01, ps01, ps23, ps23]
                            for ipass in range(6):  # (hi,lo) x kx
                                wap = ww_t[0:108, 64 * ipass:64 * ipass + 64]
                                kx = ipass % 3
                                first = ipass == 0
                                last = ipass == 5
                                for hb in range(HB):
                                    ps = pst[hb]
                                    half = hb % 2
                                    ss_r = ss_ts[hb][:].rearrange(
                                        "k (r c) -> k r c", c=WP)
                                    nr, isl = 0, 0
                                    while nr < cr:
                                        srr = min(SR, cr - nr)
                                        out_ap = ps[64 * half:64 * half + 64,
                                                    isl * 512:
                                                    isl * 512 + srr * W]
                                        nc.tensor.matmul(
                                            out_ap, wap,
                                            ss_r[0:108,
                                                 lr0 + nr:lr0 + nr + srr,
                                                 kx:kx + W],
                                            start=first, stop=last,
                                            skip_group_check=True)
                                        nr += srr
                                        isl += 1
                            # extraction: gate=sigmoid(ps+bg), cur=ps+bc
                            coff = (r0 - sr0) * W
                            bgap = bias_t[0:32, 0:1]
                            bcap = bias_t[32:64, 0:1]
                            for hb in range(HB):
                                ps = pst[hb]
                                h0 = 64 * (hb % 2)
                                if cr == nsl * SR:
                                    ps_g = ps[h0:h0 + 32, :].rearrange(
                                        "p (n x) -> p n x", x=512)[
                                        :, 0:nsl, 0:SR * W]
                                    ps_c = ps[h0 + 32:h0 + 64, :].rearrange(
                                        "p (n x) -> p n x", x=512)[
                                        :, 0:nsl, 0:SR * W]
                                    go = gt[32 * hb:32 * (hb + 1),
                                            coff:coff + F].rearrange(
                                        "p (n x) -> p n x", x=SR * W)
                                    co = ct[32 * hb:32 * (hb + 1),
                                            coff:coff + F].rearrange(
                                        "p (n x) -> p n x", x=SR * W)
                                    nc.scalar.activation(go, ps_g,
                                                         AF.Sigmoid,
                                                         bias=bgap)
                                    nc.scalar.activation(co, ps_c,
                                                         AF.Identity,
                                                         bias=bcap)
                                else:
                                    nr, isl = 0, 0
                                    while nr < cr:
                                        srr = min(SR, cr - nr)
                                        o0, o1 = (coff + nr * W,
                                                  coff + (nr + srr) * W)
                                        p0 = isl * 512
                                        nc.scalar.activation(
                                            gt[32 * hb:32 * (hb + 1),
                                               o0:o1],
                                            ps[h0:h0 + 32,
                                               p0:p0 + srr * W],
                                            AF.Sigmoid, bias=bgap)
                                        nc.scalar.activation(
                                            ct[32 * hb:32 * (hb + 1),
                                               o0:o1],
                                            ps[h0 + 32:h0 + 64,
                                               p0:p0 + srr * W],
                                            AF.Identity, bias=bcap)
                                        nr += srr
                                        isl += 1

                        # ---- state phase for this state chunk ----
                        F = srows * W
                        sl = slice(sr0 * W, sr0 * W + F)
                        v_t = stpool.tile([128, SFMAX], F32, tag="v",
                                          name="v_t")
                        nc.vector.tensor_tensor(v_t[:, :F], gt[:, :F],
                                                vmem[:, sl], OP.mult)
                        nc.vector.tensor_tensor(v_t[:, :F], v_t[:, :F],
                                                ct[:, :F], OP.add)
                        # spikes: +-1 via ACT Sign(v-0.5), strided write
                        spk_sl = spkw[:].rearrange("p (r c) -> p r c",
                                                   c=WP)[
                            :, sr0 + 1:sr0 + 1 + srows, 1:1 + W]
                        nc.scalar.activation(spk_sl,
                                             v_t[:, :F].rearrange(
                                                 "p (r c) -> p r c", c=W),
                                             AF.Sign,
                                             bias=bias_t[:, 1:2])
                        # u = -0.25*pm - 0.25  (= -0.5*spike)
                        u_t = ukpool.tile([128, SFMAX], F16, tag="u",
                                          name="u_t")
                        nc.scalar.activation(u_t[:, :F].rearrange(
                            "p (r c) -> p r c", c=W), spk_sl,
                            AF.Copy, bias=-0.25, scale=-0.25)
                        # S += v
                        nc.vector.tensor_tensor(S_t[:, sl], S_t[:, sl],
                                                v_t[:, :F], OP.add)
                        # key = seg - u (= seg + 0.5*spike)
                        key_t = ukpool.tile([128, SFMAX], F16, tag="key",
                                            name="key_t")
                        nc.vector.tensor_tensor(key_t[:, :F], seg[:, sl],
                                                u_t[:, :F], OP.subtract)
                        # vmem = v + u ; seg = key - u   (Pool engine)
                        nc.gpsimd.tensor_tensor(vmem[:, sl], v_t[:, :F],
                                                u_t[:, :F], OP.add)
                        nc.gpsimd.tensor_tensor(seg[:, sl], key_t[:, :F],
                                                u_t[:, :F], OP.subtract)
                        # captures
                        for s in range(min(t + 1, TSEG)):
                            m_t = mpool.tile([128, SFMAX], mybir.dt.uint16,
                                             tag="m", name="m_t")
                            nc.vector.tensor_scalar(m_t[:, :F], key_t[:, :F],
                                                    s + 0.5, None,
                                                    OP.is_equal)
                            nc.vector.copy_predicated(C_ts[s][:, sl],
                                                      m_t[:, :F],
                                                      S_t[:, sl])
                if debug and b == 0:
                    dbgt = gcpool.tile([128, SFMAX], F32, tag="gate",
                                       name="dbgt")
                    nc.vector.tensor_copy(out=dbgt[:, :NPIX], in_=vmem[:])
                    nc.sync.dma_start(out=dbg_d[0], in_=dbgt[:, :NPIX])
                    nc.vector.tensor_copy(out=dbgt[:, :NPIX], in_=S_t[:])
                    nc.sync.dma_start(out=dbg_d[1], in_=dbgt[:, :NPIX])
                    nc.vector.tensor_copy(out=dbgt[:, :NPIX], in_=seg[:])
                    nc.sync.dma_start(out=dbg_d[2], in_=dbgt[:, :NPIX])
                    nc.vector.tensor_copy(out=dbgt[:, :NPIX],
                                          in_=C_ts[0][:])
                    nc.sync.dma_start(out=dbg_d[3], in_=dbgt[:, :NPIX])

                # ---- final flush + diffs + output ----
                for (sr0, srows) in schunks:
                    F = srows * W
                    sl = slice(sr0 * W, sr0 * W + F)
                    for s in range(TSEG):
                        mf = mpool.tile([128, SFMAX], mybir.dt.uint16,
                                        tag="m", name="mf")
                        nc.vector.tensor_scalar(mf[:, :F], seg[:, sl],
                                                s + 0.5, None, OP.is_lt)
                        nc.vector.copy_predicated(C_ts[s][:, sl],
                                                  mf[:, :F], S_t[:, sl])
                    for s in range(TSEG):
                        og = gcpool.tile([128, SFMAX], F32, tag="gate",
                                         name="og")
                        if s == 0:
                            nc.vector.tensor_copy(out=og[:, :F],
                                                  in_=C_ts[0][:, sl])
                        else:
                            nc.vector.tensor_tensor(og[:, :F],
                                                    C_ts[s][:, sl],
                                                    C_ts[s - 1][:, sl],
                                                    OP.subtract)
                        o4 = out_d[s, b].rearrange(
                            "c (r i) w -> c r i w", i=HB)
                        for hb in range(HB):
                            nc.sync.dma_start(
                                out=o4[:, sr0:sr0 + srows, hb, :],
                                in_=og[32 * hb:32 * (hb + 1), :F])
    _split_matmul_waits(nc)
    return nc


def _split_matmul_waits(nc):
    """Walrus's LDW+MATMUL pair (and 2D DMA descriptors) have a single
    sync-wait slot; move extra waits onto same-engine no-ops inserted just
    before the instruction (safe: waits execute in order on the sequencer)."""
    nid = [0]
    for blk in nc.m.functions[0].blocks:
        out = []
        for inst in blk.instructions:
            si = inst.sync_info
            if (type(inst).__name__ != 'InstNoOp' and si is not None
                    and len(si.on_wait) > 1):
                keep = si.on_wait[-1:]
                for w in si.on_wait[:-1]:
                    nop = mybir.InstNoOp(name=f"NW-{nid[0]}", ins=[], outs=[])
                    nid[0] += 1
                    nop.engine = inst.engine
                    zupd = mybir.SyncUpdate(
                        sync_type='semaphore', id=nc.nop_sem_num,
                        ant_name='nopsem', update_mode='sem-inc',
                        update_value=1, update_reg=None)
                    nop.sync_info = mybir.SyncInfo(on_wait=[w],
                                                   on_update=[zupd])
                    out.append(nop)
                inst.sync_info = mybir.SyncInfo(on_wait=keep,
                                                on_update=si.on_update)
            out.append(inst)
        blk.instructions = out


def host_prep(events, w_in, b_in, w_gate, b_gate, ncores=NCORES):
    """Build per-core input maps. events: [B,T,CIN,H,W] full."""
    Bf, Tf, Cf, Hf, Wf = events.shape
    Cout2 = w_gate.shape[0]          # 64
    BR = Hf // HB
    WP = Wf + 2
    evr = np.ascontiguousarray(events[:, ::-1]).astype(np.float32)
    evh = evr.astype(np.float16)
    evl = (evr - evh.astype(np.float32)).astype(np.float16)
    # padded planes [B,T,2,H+2,WP]
    def padp(x):
        p = np.zeros((Bf, Tf, Cf, Hf + 2, WP), np.float16)
        p[..., 1:1 + Hf, 1:1 + Wf] = x
        return p
    evph, evpl = padp(evh), padp(evl)
    # ev_d [B,T,4,12,BR*WP]: row hl*6+ky*2+cin content r -> pad[g0+r+ky]
    ev_st = np.zeros((Bf, Tf, HB, 12, BR, WP), np.float16)
    for hl, srcp in ((0, evph), (1, evpl)):
        for ky in range(3):
            for cin in range(Cf):
                for hb in range(HB):
                    # local row r of block hb taps global row 4r+hb+ky-1
                    # = padded row 4r+hb+ky
                    ev_st[:, :, hb, hl * 6 + ky * 2 + cin] = \
                        srcp[:, :, cin, hb + ky:hb + ky + 4 * BR:4, :]
    ev_st = ev_st.reshape(Bf, Tf, HB, 12, BR * WP)

    # weights: 6 tiles [108,64] packed in ww[128,384]
    wg = 0.5 * np.asarray(w_gate, np.float32)
    wgh = wg.astype(np.float16)
    wgl = (wg - wgh.astype(np.float32)).astype(np.float16)
    wi = np.asarray(w_in, np.float32)
    wih = wi.astype(np.float16)
    wil = (wi - wih.astype(np.float32)).astype(np.float16)
    ww = np.zeros((128, 384), np.float16)
    for ipass in range(6):
        hi = ipass < 3
        kx = ipass % 3
        c0 = 64 * ipass
        wgp = wgh if hi else wgl
        we1 = wih if hi else wil   # on ev_hi rows
        we2 = wil if hi else wih   # on ev_lo rows (cross)
        for ky in range(3):
            for c in range(COUT):
                ww[ky * 32 + c, c0:c0 + 64] = wgp[:, c, ky, kx]
            for cin in range(Cf):
                ww[96 + ky * 2 + cin, c0:c0 + 64] = we1[:, cin, ky, kx]
                ww[102 + ky * 2 + cin, c0:c0 + 64] = we2[:, cin, ky, kx]

    # bias: b + 0.5*sum(w_gate) per out channel; rows [bg,bc,bg,bc]x32
    bsum = 0.5 * np.asarray(w_gate, np.float32).sum(axis=(1, 2, 3))
    beff = (np.asarray(b_gate, np.float32) + np.asarray(b_in, np.float32)
            + bsum)                       # [64]
    bias = np.zeros((128, 2), np.float32)
    bias[:, 1] = -THRESH
    bias[0:32, 0] = beff[:32]
    bias[32:64, 0] = beff[32:]
    bias[64:96, 0] = beff[:32]
    bias[96:128, 0] = beff[32:]

    bl = Bf // ncores
    in_maps = []
    for i in range(ncores):
        ev_i = ev_st[i * bl:(i + 1) * bl].reshape(bl * Tf, HB, 12, BR * WP)
        in_maps.append({"ev": np.ascontiguousarray(ev_i), "ww": ww,
                        "bias": bias})
    return in_maps


_cache = {}
last_run_info = {}


def kernel(events, w_in, b_in, w_gate, b_gate, trace=False):
    from concourse import bass_utils
    key = ("v2",)
    if key not in _cache:
        _cache[key] = build_nc()
    nc = _cache[key]
    in_maps = host_prep(np.asarray(events), np.asarray(w_in),
                        np.asarray(b_in), np.asarray(w_gate),
                        np.asarray(b_gate))
    t0 = time.time()
    res = bass_utils.run_bass_kernel_spmd(
        nc, in_maps, core_ids=list(range(NCORES)), trace=trace)
    wall = time.time() - t0
    last_run_info.update(exec_time_ns=res.exec_time_ns, wall_s=wall,
                         profile_json=getattr(res, "profile_json", None))
    outs = [res.results[i]["out"] for i in range(NCORES)]
    return np.concatenate(outs, axis=1)


# revision 18
# speedup vs baseline: 2.3485x; 1.0272x over previous
"""AdaptiveRSNNEmbedding Trainium2 kernel (8 NeuronCores, batch-parallel).

Reference semantics (per batch element, T time-reversed steps):
    g, c   = split(conv3x3(spike_prev, w_gate) + conv3x3(ev_t, w_in) + biases)
    gate   = sigmoid(g);  v = gate*vmem + c
    spike  = (v > 0.5);   vmem' = v - 0.5*spike
    agg[seg] accumulates vavg at spikes (seg<4), seg += spike, plus a final
    flush of the unclosed segment.

v2 design:
  * agg via cumulative-sum captures: S_t = sum v_tau; C[s] = S at the spike
    closing segment s; final C'[s] = (seg>s) ? C[s] : S_end;
    agg[s] = C'[s]-C'[s-1]. Captures are copy_predicated writes (4/step).
  * spikes stored as +-1 (ACT Sign(v-0.5)) with -1 padding; conv identity
    conv(h) = 0.5*conv(pm) + 0.5*sum(w) makes interior AND edges exact with
    the constant folded into the per-channel extraction bias.
  * conv in fp16 hi+lo passes (exact to ~22 bits): per kx, mm_hi uses
    [wg_hi | wi_hi(ev_hi rows) | wi_lo(ev_lo rows)] and mm_lo uses
    [wg_lo | wi_lo(ev_hi rows) | wi_hi(ev_lo rows)] so the event product
    (ev_hi+ev_lo)*(wi_hi+wi_lo) is complete. Events ride as 12 extra K rows
    (2cin x 3ky x hi/lo) in padded spatial layout, ky pre-shifted on host.
  * layout: partition p = 32*hb + c (hb = H/4 row block), free = r*W+x.
    psum pair tiles [128, nsl*512] hold 2 row blocks (hb pair) per chunk.
  * state updates at 2-conv-chunk granularity; vmem/seg on Pool engine;
    S/C/seg/masks/spk in fp16 (exact or validated), v path in f32.
"""
import sys
import time
import numpy as np

if '/opt/trn_rl_repo' not in sys.path:
    sys.path.insert(0, '/opt/trn_rl_repo')

import concourse.bass as bass
import concourse.mybir as mybir
from concourse.tile import TileContext

F32 = mybir.dt.float32
F16 = mybir.dt.float16
AF = mybir.ActivationFunctionType
OP = mybir.AluOpType

B, T, CIN, COUT, H, W = 16, 16, 2, 32, 160, 160
TSEG = 4
NCORES = 8
BL = B // NCORES
THRESH = 0.5
HB = 4


def _chunks(BR, CR):
    out = []
    r = 0
    while r < BR:
        cr = min(CR, BR - r)
        out.append((r, cr))
        r += cr
    return out


def build_nc(BL=BL, T=T, H=H, W=W, debug=False):
    BR = H // HB
    NPIX = BR * W
    WP = W + 2
    SR = max(1, 512 // W)          # rows per psum slice
    CRMAX = 2 * SR                 # conv chunk rows
    cchunks = _chunks(BR, CRMAX)
    # state chunks = pairs of conv chunks
    schunks = []
    i = 0
    while i < len(cchunks):
        r0 = cchunks[i][0]
        rows = cchunks[i][1]
        if i + 1 < len(cchunks) and cchunks[i + 1][1] == CRMAX:
            rows += cchunks[i + 1][1]
            i += 2
        else:
            i += 1
        schunks.append((r0, rows))
    SFMAX = max(rows for _, rows in schunks) * W

    nc = bass.Bass()
    nop_sem = nc.semaphore("nopsem").__enter__()
    ev_d = nc.declare_dram_parameter("ev", [BL * T, HB, 12, BR * WP], F16,
                                     isOutput=False)
    ww_d = nc.declare_dram_parameter("ww", [128, 384], F16, isOutput=False)
    bias_d = nc.declare_dram_parameter("bias", [128, 2], F32, isOutput=False)
    out_d = nc.declare_dram_parameter("out", [TSEG, BL, COUT, H, W], F32,
                                      isOutput=True)
    if debug:
        dbg_d = nc.declare_dram_parameter("dbg", [4, 128, (H // HB) * W],
                                          F32, isOutput=True)

    with TileContext(nc) as tc:
        with tc.tile_pool(name="const", bufs=1) as cpool, \
             tc.tile_pool(name="state", bufs=1) as spool, \
             tc.tile_pool(name="wss", bufs=2) as sspool, \
             tc.tile_pool(name="wgc", bufs=1) as gcpool, \
             tc.tile_pool(name="wst", bufs=2) as stpool, \
             tc.tile_pool(name="wuk", bufs=1) as ukpool, \
             tc.tile_pool(name="wm", bufs=2) as mpool, \
             tc.tile_pool(name="psum", bufs=2, space="PSUM") as ppool:

            ww_t = cpool.tile([128, 384], F16, tag="ww")
            nc.sync.dma_start(out=ww_t[:], in_=ww_d[:])
            bias_t = cpool.tile([128, 2], F32, tag="bias")
            nc.sync.dma_start(out=bias_t[:], in_=bias_d[:])

            vmem = spool.tile([128, NPIX], F32, tag="vmem")
            S_t = spool.tile([128, NPIX], F16, tag="S")
            seg = spool.tile([128, NPIX], F16, tag="seg")
            C_ts = [spool.tile([128, NPIX], F16, tag=f"C{s}", name=f"C{s}")
                    for s in range(TSEG)]
            spkbufs = [spool.tile([128, (BR + 2) * WP], F16, tag=f"spk{i}",
                                  name=f"spk{i}") for i in range(2)]

            nc.nop_sem_num = nop_sem.num

            for b in range(BL):
                # zero/reset state
                nc.vector.memset(vmem[:], 0.0)
                nc.vector.memset(S_t[:], 0.0)
                nc.vector.memset(seg[:], 0.0)
                for s in range(TSEG):
                    nc.gpsimd.memset(C_ts[s][:], 0.0)
                nc.gpsimd.memset(spkbufs[0][:], -1.0)
                nc.gpsimd.memset(spkbufs[1][:], -1.0)

                for t in range(T):
                    spk = spkbufs[t % 2]
                    spkw = spkbufs[(t + 1) % 2]
                    bt = b * T + t

                    for (sr0, srows) in schunks:
                        # ---- conv phase for this state chunk ----
                        gt = gcpool.tile([128, SFMAX], F32, tag="gate",
                                         name="gt")
                        ct = gcpool.tile([128, SFMAX], F32, tag="cur",
                                         name="ct")
                        # stack spikes + events per hb for the whole sc
                        SCR = SFMAX // W
                        ss_ts = []
                        for hb in range(HB):
                            ss = sspool.tile([128, SCR * WP], F16,
                                             tag=f"ss{hb}", name=f"ss{hb}")
                            ss_ts.append(ss)
                            for ky in range(3):
                                hs = (hb + ky - 1) % HB
                                dlt = (hb + ky - 1) // HB
                                nc.sync.dma_start(
                                    out=ss[32 * ky:32 * (ky + 1),
                                           :srows * WP],
                                    in_=spk[32 * hs:32 * (hs + 1),
                                            (sr0 + dlt + 1) * WP:
                                            (sr0 + dlt + 1 + srows) * WP])
                            nc.sync.dma_start(
                                out=ss[96:108, :srows * WP],
                                in_=ev_d[bt, hb, :,
                                         sr0 * WP:(sr0 + srows) * WP])
                        for (r0, cr) in [c for c in cchunks
                                         if sr0 <= c[0] < sr0 + srows]:
                            F = cr * W
                            nsl = (cr + SR - 1) // SR
                            lr0 = r0 - sr0
                            ps01 = ppool.tile([128, 1024], F32,
                                              tag="ps01", name="ps01")
                            ps23 = ppool.tile([128, 1024], F32,
                                              tag="ps23", name="ps23")
                            pst = [ps01, ps01, ps23, ps23]
                            for ipass in range(6):  # (hi,lo) x kx
                                wap = ww_t[0:108, 64 * ipass:64 * ipass + 64]
                                kx = ipass % 3
                                first = ipass == 0
                                last = ipass == 5
                                for hb in range(HB):
                                    ps = pst[hb]
                                    half = hb % 2
                                    ss_r = ss_ts[hb][:].rearrange(
                                        "k (r c) -> k r c", c=WP)
                                    nr, isl = 0, 0
                                    while nr < cr:
                                        srr = min(SR, cr - nr)
                                        out_ap = ps[64 * half:64 * half + 64,
                                                    isl * 512:
                                                    isl * 512 + srr * W]
                                        nc.tensor.matmul(
                                            out_ap, wap,
                                            ss_r[0:108,
                                                 lr0 + nr:lr0 + nr + srr,
                                                 kx:kx + W],
                                            start=first, stop=last,
                                            skip_group_check=True)
                                        nr += srr
                                        isl += 1
                            # extraction: gate=sigmoid(ps+bg), cur=ps+bc
                            coff = (r0 - sr0) * W
                            bgap = bias_t[0:32, 0:1]
                            bcap = bias_t[32:64, 0:1]
                            for hb in range(HB):
                                ps = pst[hb]
                                h0 = 64 * (hb % 2)
                                if cr == nsl * SR:
                                    ps_g = ps[h0:h0 + 32, :].rearrange(
                                        "p (n x) -> p n x", x=512)[
                                        :, 0:nsl, 0:SR * W]
                                    ps_c = ps[h0 + 32:h0 + 64, :].rearrange(
                                        "p (n x) -> p n x", x=512)[
                                        :, 0:nsl, 0:SR * W]
                                    go = gt[32 * hb:32 * (hb + 1),
                                            coff:coff + F].rearrange(
                                        "p (n x) -> p n x", x=SR * W)
                                    co = ct[32 * hb:32 * (hb + 1),
                                            coff:coff + F].rearrange(
                                        "p (n x) -> p n x", x=SR * W)
                                    nc.scalar.activation(go, ps_g,
                                                         AF.Sigmoid,
                                                         bias=bgap)
                                    nc.scalar.activation(co, ps_c,
                                                         AF.Identity,
                                                         bias=bcap)
                                else:
                                    nr, isl = 0, 0
                                    while nr < cr:
                                        srr = min(SR, cr - nr)
                                        o0, o1 = (coff + nr * W,
                                                  coff + (nr + srr) * W)
                                        p0 = isl * 512
                                        nc.scalar.activation(
                                            gt[32 * hb:32 * (hb + 1),
                                               o0:o1],
                                            ps[h0:h0 + 32,
                                               p0:p0 + srr * W],
                                            AF.Sigmoid, bias=bgap)
                                        nc.scalar.activation(
                                            ct[32 * hb:32 * (hb + 1),
                                               o0:o1],
                                            ps[h0 + 32:h0 + 64,
                                               p0:p0 + srr * W],
                                            AF.Identity, bias=bcap)
                                        nr += srr
                                        isl += 1

                        # ---- state phase for this state chunk ----
                        F = srows * W
                        sl = slice(sr0 * W, sr0 * W + F)
                        v_t = stpool.tile([128, SFMAX], F32, tag="v",
                                          name="v_t")
                        nc.vector.tensor_tensor(v_t[:, :F], gt[:, :F],
                                                vmem[:, sl], OP.mult)
                        nc.vector.tensor_tensor(v_t[:, :F], v_t[:, :F],
                                                ct[:, :F], OP.add)
                        # spikes: +-1 via ACT Sign(v-0.5), strided write
                        spk_sl = spkw[:].rearrange("p (r c) -> p r c",
                                                   c=WP)[
                            :, sr0 + 1:sr0 + 1 + srows, 1:1 + W]
                        nc.scalar.activation(spk_sl,
                                             v_t[:, :F].rearrange(
                                                 "p (r c) -> p r c", c=W),
                                             AF.Sign,
                                             bias=bias_t[:, 1:2])
                        # u = -0.25*pm - 0.25  (= -0.5*spike)
                        u_t = ukpool.tile([128, SFMAX], F16, tag="u",
                                          name="u_t")
                        nc.vector.tensor_scalar(
                            u_t[:, :F].rearrange("p (r c) -> p r c", c=W),
                            spk_sl, -0.25, -0.25, OP.mult, OP.add)
                        # S += v  (Pool engine)
                        nc.gpsimd.tensor_tensor(S_t[:, sl], S_t[:, sl],
                                                v_t[:, :F], OP.add)
                        # key = seg - u (= seg + 0.5*spike)
                        key_t = ukpool.tile([128, SFMAX], F16, tag="key",
                                            name="key_t")
                        nc.vector.tensor_tensor(key_t[:, :F], seg[:, sl],
                                                u_t[:, :F], OP.subtract)
                        # vmem = v + u ; seg = key - u   (Pool engine)
                        nc.gpsimd.tensor_tensor(vmem[:, sl], v_t[:, :F],
                                                u_t[:, :F], OP.add)
                        nc.gpsimd.tensor_tensor(seg[:, sl], key_t[:, :F],
                                                u_t[:, :F], OP.subtract)
                        # captures
                        for s in range(min(t + 1, TSEG)):
                            m_t = mpool.tile([128, SFMAX], mybir.dt.uint16,
                                             tag="m", name="m_t")
                            nc.vector.tensor_scalar(m_t[:, :F], key_t[:, :F],
                                                    s + 0.5, None,
                                                    OP.is_equal)
                            nc.vector.copy_predicated(C_ts[s][:, sl],
                                                      m_t[:, :F],
                                                      S_t[:, sl])
                if debug and b == 0:
                    dbgt = gcpool.tile([128, SFMAX], F32, tag="gate",
                                       name="dbgt")
                    nc.vector.tensor_copy(out=dbgt[:, :NPIX], in_=vmem[:])
                    nc.sync.dma_start(out=dbg_d[0], in_=dbgt[:, :NPIX])
                    nc.vector.tensor_copy(out=dbgt[:, :NPIX], in_=S_t[:])
                    nc.sync.dma_start(out=dbg_d[1], in_=dbgt[:, :NPIX])
                    nc.vector.tensor_copy(out=dbgt[:, :NPIX], in_=seg[:])
                    nc.sync.dma_start(out=dbg_d[2], in_=dbgt[:, :NPIX])
                    nc.vector.tensor_copy(out=dbgt[:, :NPIX],
                                          in_=C_ts[0][:])
                    nc.sync.dma_start(out=dbg_d[3], in_=dbgt[:, :NPIX])

                # ---- final flush + diffs + output ----
                for (sr0, srows) in schunks:
                    F = srows * W
                    sl = slice(sr0 * W, sr0 * W + F)
                    for s in range(TSEG):
                        mf = mpool.tile([128, SFMAX], mybir.dt.uint16,
                                        tag="m", name="mf")
                        nc.vector.tensor_scalar(mf[:, :F], seg[:, sl],
                                                s + 0.5, None, OP.is_lt)
                        nc.vector.copy_predicated(C_ts[s][:, sl],
                                                  mf[:, :F], S_t[:, sl])
                    for s in range(TSEG):
                        og = gcpool.tile([128, SFMAX], F32, tag="gate",
                                         name="og")
                        if s == 0:
                            nc.vector.tensor_copy(out=og[:, :F],
                                                  in_=C_ts[0][:, sl])
                        else:
                            nc.vector.tensor_tensor(og[:, :F],
                                                    C_ts[s][:, sl],
                                                    C_ts[s - 1][:, sl],
                                                    OP.subtract)
                        o4 = out_d[s, b].rearrange(
                            "c (r i) w -> c r i w", i=HB)
                        for hb in range(HB):
                            nc.sync.dma_start(
                                out=o4[:, sr0:sr0 + srows, hb, :],
                                in_=og[32 * hb:32 * (hb + 1), :F])
    _split_matmul_waits(nc)
    return nc


def _split_matmul_waits(nc):
    """Walrus's LDW+MATMUL pair (and 2D DMA descriptors) have a single
    sync-wait slot; move extra waits onto same-engine no-ops inserted just
    before the instruction (safe: waits execute in order on the sequencer)."""
    nid = [0]
    for blk in nc.m.functions[0].blocks:
        out = []
        for inst in blk.instructions:
            si = inst.sync_info
            if (type(inst).__name__ != 'InstNoOp' and si is not None
                    and len(si.on_wait) > 1):
                keep = si.on_wait[-1:]
                for w in si.on_wait[:-1]:
                    nop = mybir.InstNoOp(name=f"NW-{nid[0]}", ins=[], outs=[])
                    nid[0] += 1
                    nop.engine = inst.engine
                    zupd = mybir.SyncUpdate(
                        sync_type='semaphore', id=nc.nop_sem_num,
                        ant_name='nopsem', update_mode='sem-inc',
                        update_value=1, update_reg=None)
                    nop.sync_info = mybir.SyncInfo(on_wait=[w],
                                                   on_update=[zupd])
                    out.append(nop)
                inst.sync_info = mybir.SyncInfo(on_wait=keep,
                                                on_update=si.on_update)
            out.append(inst)
        blk.instructions = out


def host_prep(events, w_in, b_in, w_gate, b_gate, ncores=NCORES):
    """Build per-core input maps. events: [B,T,CIN,H,W] full."""
    Bf, Tf, Cf, Hf, Wf = events.shape
    Cout2 = w_gate.shape[0]          # 64
    BR = Hf // HB
    WP = Wf + 2
    evr = np.ascontiguousarray(events[:, ::-1]).astype(np.float32)
    evh = evr.astype(np.float16)
    evl = (evr - evh.astype(np.float32)).astype(np.float16)
    # padded planes [B,T,2,H+2,WP]
    def padp(x):
        p = np.zeros((Bf, Tf, Cf, Hf + 2, WP), np.float16)
        p[..., 1:1 + Hf, 1:1 + Wf] = x
        return p
    evph, evpl = padp(evh), padp(evl)
    # ev_d [B,T,4,12,BR*WP]: row hl*6+ky*2+cin content r -> pad[g0+r+ky]
    ev_st = np.zeros((Bf, Tf, HB, 12, BR, WP), np.float16)
    for hl, srcp in ((0, evph), (1, evpl)):
        for ky in range(3):
            for cin in range(Cf):
                for hb in range(HB):
                    # local row r of block hb taps global row 4r+hb+ky-1
                    # = padded row 4r+hb+ky
                    ev_st[:, :, hb, hl * 6 + ky * 2 + cin] = \
                        srcp[:, :, cin, hb + ky:hb + ky + 4 * BR:4, :]
    ev_st = ev_st.reshape(Bf, Tf, HB, 12, BR * WP)

    # weights: 6 tiles [108,64] packed in ww[128,384]
    wg = 0.5 * np.asarray(w_gate, np.float32)
    wgh = wg.astype(np.float16)
    wgl = (wg - wgh.astype(np.float32)).astype(np.float16)
    wi = np.asarray(w_in, np.float32)
    wih = wi.astype(np.float16)
    wil = (wi - wih.astype(np.float32)).astype(np.float16)
    ww = np.zeros((128, 384), np.float16)
    for ipass in range(6):
        hi = ipass < 3
        kx = ipass % 3
        c0 = 64 * ipass
        wgp = wgh if hi else wgl
        we1 = wih if hi else wil   # on ev_hi rows
        we2 = wil if hi else wih   # on ev_lo rows (cross)
        for ky in range(3):
            for c in range(COUT):
                ww[ky * 32 + c, c0:c0 + 64] = wgp[:, c, ky, kx]
            for cin in range(Cf):
                ww[96 + ky * 2 + cin, c0:c0 + 64] = we1[:, cin, ky, kx]
                ww[102 + ky * 2 + cin, c0:c0 + 64] = we2[:, cin, ky, kx]

    # bias: b + 0.5*sum(w_gate) per out channel; rows [bg,bc,bg,bc]x32
    bsum = 0.5 * np.asarray(w_gate, np.float32).sum(axis=(1, 2, 3))
    beff = (np.asarray(b_gate, np.float32) + np.asarray(b_in, np.float32)
            + bsum)                       # [64]
    bias = np.zeros((128, 2), np.float32)
    bias[:, 1] = -THRESH
    bias[0:32, 0] = beff[:32]
    bias[32:64, 0] = beff[32:]
    bias[64:96, 0] = beff[:32]
    bias[96:128, 0] = beff[32:]

    bl = Bf // ncores
    in_maps = []
    for i in range(ncores):
        ev_i = ev_st[i * bl:(i + 1) * bl].reshape(bl * Tf, HB, 12, BR * WP)
        in_maps.append({"ev": np.ascontiguousarray(ev_i), "ww": ww,
                        "bias": bias})
    return in_maps


_cache = {}
last_run_info = {}


def kernel(events, w_in, b_in, w_gate, b_gate, trace=False):
    from concourse import bass_utils
    key = ("v2",)
    if key not in _cache:
        _cache[key] = build_nc()
    nc = _cache[key]
    in_maps = host_prep(np.asarray(events), np.asarray(w_in),
                        np.asarray(b_in), np.asarray(w_gate),
                        np.asarray(b_gate))
    t0 = time.time()
    res = bass_utils.run_bass_kernel_spmd(
        nc, in_maps, core_ids=list(range(NCORES)), trace=trace)
    wall = time.time() - t0
    last_run_info.update(exec_time_ns=res.exec_time_ns, wall_s=wall,
                         profile_json=getattr(res, "profile_json", None))
    outs = [res.results[i]["out"] for i in range(NCORES)]
    return np.concatenate(outs, axis=1)
